# revision 46
# baseline (speedup 1.0000x reference)
"""Trainium2 Bass kernel for the 4-layer cross-stencil CNN.

Per-core: one image [6,256,256] (batch sharded across 8 cores).

Line-buffer pipeline with circular row slots (no halo recompute) and
fp8e4m3 DoubleRow matmuls (0.5 cyc/elem) for the two 128->128 layers:

  h stored as fp8 (main, resid) plane pair; per tap one DR matmul
  (Whi_t, Whi_t) x (h_m@t, h_r@t) gives Whi_t.h exactly-compensated in
  the activations, plus 4 DR matmuls for the Wlo = W - Whi correction
  (pair strides must be multiples of 16B, so left/right taps ride
  half-empty plane-pair DRs).  Residual plane is one fused DVE op:
  r = (psum max 0) - m.

L1 runs bf16 with the 5 taps packed on K=30 partitions (pre-shifted
x copies; out-of-range edge slots zeroed so edge rows use the same
packed matmul).  L4 = one bf16 slab matmul (all 5 tap matrices as
M=128 output slabs) + the tap sum done by 5 accumulating gather DMAs
(gpsimd, cast bf16->f32, 3-D APs to keep descriptor runs ~1KB), so no
selector matmul.  Bias enters via the activation copies (center slab
for b4).

Line-buffer schedule: circular row slots (h1/h2/h3 S=44, x30 SX=72),
per-strip stage targets skewed (+6/+4/+2/+1 rows) so every conv pair's
inputs exist within its strip frame; LAG rows of emission slack hide
the PSUM->SBUF copy latency; per-2-row shared PSUM tiles halve the
fixed PSUM access penalties on ACT/DVE.  KSTAGES env var (default:
everything on) is a debug knob that drops later pipeline stages.
"""

import os
import sys

sys.path.insert(0, "/opt/trn_rl_repo")

KSTAGES = int(os.environ.get("KSTAGES", "9"))

import ml_dtypes
import numpy as np

import concourse.bacc as bacc
import concourse.mybir as mybir
from concourse.tile import TileContext
from concourse import bass_utils

IN_C, HID_C, OUT_C = 6, 128, 6
B, H, W = 8, 256, 256
N_CORES = 8

R = 32          # output rows per strip
NS = H // R     # strips
WP = 272        # padded row pitch (multiple of 16 for DR stride rules)
C0 = 8          # data col d lives at padded col C0+d
S = 48          # circular row slots for h1/h2/h3
LAG = 6         # extra pipeline rows between stages (hides copy latency)
SX = 72         # circular row slots for x30 (covers one-strip prefetch)
T5R = R + 2     # strip-local t5 rows [a-1, b+1)

SW = 32.0       # fp8 weight scale for L2/L3 (2^5)

f32 = mybir.dt.float32
bf16 = mybir.dt.bfloat16
f8 = mybir.dt.float8e4
Add = mybir.AluOpType.add
Sub = mybir.AluOpType.subtract
Max = mybir.AluOpType.max
Bypass = mybir.AluOpType.bypass
Relu = mybir.ActivationFunctionType.Relu
Ident = mybir.ActivationFunctionType.Identity
DR = mybir.MatmulPerfMode.DoubleRow

# taps: (name, dr, dc); x30/L1 group order u,c,l,r,d
TAPS = [("u", -1, 0), ("c", 0, 0), ("l", 0, -1), ("r", 0, 1), ("d", 1, 0)]
# weight-group indices in the packed [128, 2, 11, 128] fp8 tensor
G_MAIN = {"u": 0, "c": 1, "l": 2, "r": 3, "d": 4}
G_WLO_A = 5   # (Wlo_u, Wlo_c), rhs (hm@u, hm@c) stride WP
G_WLO_B = 6   # (Wlo_d, 0)
G_WLO_C = 7   # (Wlo_l, 0)
G_WLO_D = 8   # (Wlo_r, 0)
G_WLO_CS = 9  # (Wlo_c, 0)
G_WLO_US = 10  # (Wlo_u, 0)

# t5 slab base partitions (tap -> partition)
SLAB = {"u": 0, "c": 6, "d": 32, "l": 64, "r": 96}


def _build(has_bias=False):
    nc = bacc.Bacc("TRN2", target_bir_lowering=False)

    x_d = nc.dram_tensor("x", [IN_C, H, W], bf16, kind="ExternalInput")
    w1p_d = nc.dram_tensor("w1p", [5 * IN_C, HID_C], bf16, kind="ExternalInput")
    w2p_d = nc.dram_tensor("w2p", [HID_C, 2, 11, HID_C], f8, kind="ExternalInput")
    w3p_d = nc.dram_tensor("w3p", [HID_C, 2, 11, HID_C], f8, kind="ExternalInput")
    w4a_d = nc.dram_tensor("w4a", [HID_C, HID_C], bf16, kind="ExternalInput")
    b1_d = nc.dram_tensor("b1", [HID_C], f32, kind="ExternalInput")
    b2s_d = nc.dram_tensor("b2s", [HID_C], f32, kind="ExternalInput")
    b3_d = nc.dram_tensor("b3", [HID_C], f32, kind="ExternalInput")
    t5b_d = nc.dram_tensor("t5b", [HID_C], f32, kind="ExternalInput")
    y_d = nc.dram_tensor("y", [OUT_C, H, W], f32, kind="ExternalOutput")

    with TileContext(nc) as tc:
        with (
            tc.tile_pool(name="const", bufs=1) as cpool,
            tc.tile_pool(name="bufs", bufs=1) as bpool,
            tc.tile_pool(name="ps1", bufs=2, space="PSUM") as ps1pool,
            tc.tile_pool(name="ps2", bufs=2, space="PSUM") as ps2pool,
            tc.tile_pool(name="ps3", bufs=2, space="PSUM") as ps3pool,
            tc.tile_pool(name="ps4", bufs=2, space="PSUM") as ps4pool,
        ):
            w1p = cpool.tile([5 * IN_C, HID_C], bf16)
            nc.gpsimd.dma_start(out=w1p, in_=w1p_d[:, :])
            w2p = cpool.tile([HID_C, 2, 11, HID_C], f8)
            nc.gpsimd.dma_start(out=w2p, in_=w2p_d[:, :, :, :])
            w3p = cpool.tile([HID_C, 2, 11, HID_C], f8)
            nc.gpsimd.dma_start(out=w3p, in_=w3p_d[:, :, :, :])
            w4a = cpool.tile([HID_C, HID_C], bf16)
            nc.gpsimd.dma_start(out=w4a, in_=w4a_d[:, :])
            b1_sb = cpool.tile([HID_C, 1], f32)
            nc.sync.dma_start(out=b1_sb, in_=b1_d[:, None])
            b2s_sb = cpool.tile([HID_C, 1], f32)
            nc.sync.dma_start(out=b2s_sb, in_=b2s_d[:, None])
            b3_sb = cpool.tile([HID_C, 1], f32)
            nc.sync.dma_start(out=b3_sb, in_=b3_d[:, None])
            t5b_sb = cpool.tile([HID_C, 1], f32)
            nc.sync.dma_start(out=t5b_sb, in_=t5b_d[:, None])

            x30 = bpool.tile([5 * IN_C, SX, WP], bf16)
            h1 = bpool.tile([HID_C, 2, S, WP], f8)
            h2 = bpool.tile([HID_C, 2, S, WP], f8)
            h3 = bpool.tile([HID_C, S, WP], bf16)
            t5a = bpool.tile([HID_C, T5R, WP], bf16)
            t5b = bpool.tile([HID_C, T5R, WP], bf16)
            t5 = [t5a, t5b]
            yacc = bpool.tile([OUT_C, T5R, WP], f32)
            if has_bias:
                zb0 = bpool.tile([HID_C, 2, W], bf16)
                zb1 = bpool.tile([HID_C, 2, W], bf16)
                zb2 = bpool.tile([HID_C, 2, W], bf16)
                zb = [zb0, zb1, zb2]

            # --- one-time zeroing of read-before-write cells ---
            # engine memsets need quadrant-aligned partition bases; use a
            # zeroed staging tile + DMA for the off-quadrant x30 bands
            zeros_bf = cpool.tile([IN_C, WP], bf16)
            nc.vector.memset(zeros_bf[:, :], 0.0)
            # x30 l-group col for x[-1]; r-group col for x[256]
            if KSTAGES >= 8:
                nc.sync.dma_start(
                    out=x30[12:18, :, C0 : C0 + 1], in_=zeros_bf[:, 0:SX])
                nc.sync.dma_start(
                    out=x30[18:24, :, C0 + 255 : C0 + 256],
                    in_=zeros_bf[:, 0:SX])
            # u-group slot for x[-1] (read by L1 row 0 only, before reuse)
            nc.vector.memset(x30[0:6, 0:1, :], 0.0)
            # h1/h2 pad+junk cols (DR rhs reads [C0-2, C0+258))
            for t in (h1, h2):
                nc.vector.memset(t[:, :, :, 0:C0], 0.0)
                nc.vector.memset(t[:, :, :, C0 + 256 : WP], 0.0)
            # t5 pads/junk cols + never-written boundary slots (gathers read
            # the full flat range)
            for t in t5:
                nc.vector.memset(t[:, :, 0:C0], 0.0)
                nc.vector.memset(t[:, :, C0 + 256 : WP], 0.0)
                nc.vector.memset(t[:, 0:1, :], 0.0)
                nc.vector.memset(t[:, T5R - 1 : T5R, :], 0.0)

            # ---------- stage emitters ----------
            def x_loads(k):
                """DMA x rows for strip k into the 5 shifted groups."""
                engs = [nc.sync]
                lo1 = 0 if k == 0 else min(H, k * R + 6)
                hi1 = min(H, k * R + R + 6)
                # (group base partition, x-row range, slot shift, col base)
                specs = [
                    (0, max(0, lo1 - 1), hi1 - 1, +1, C0),       # u
                    (6, lo1, hi1, 0, C0),                        # c
                    (12, lo1, hi1, 0, C0 + 1),                   # l
                    (18, lo1, hi1, 0, C0 - 1),                   # r
                    (24, lo1 + 1, min(H, hi1 + 1), -1, C0),      # d
                ]
                di = 0
                for p0, r0, r1, sh, cb in specs:
                    rr = r0
                    while rr < r1:
                        s0 = (rr + sh) % SX
                        n = min(r1 - rr, SX - s0)
                        engs[di % len(engs)].dma_start(
                            out=x30[p0 : p0 + 6, s0 : s0 + n, cb : cb + W],
                            in_=x_d[:, rr : rr + n, :],
                        )
                        di += 1
                        rr += n

            def emit_l1(r0, n):
                """L1 rows [r0, r0+n)."""
                if KSTAGES < 1:
                    return
                sx = r0 % SX
                s1 = r0 % S
                if (r0 == 0 or r0 + n - 1 == H - 1
                        or sx + n > SX or s1 + n > S):
                    for r in range(r0, r0 + n):
                        emit_l1_row(r)
                    return
                ps = ps1pool.tile([HID_C, n, W], f32, tag="ps")
                nc.tensor.matmul(
                    ps, w1p[:, :], x30[:, sx : sx + n, C0 : C0 + W],
                    start=True, stop=True,
                )
                l1_copies(ps, s1, n)

            def emit_l1_row(r):
                # edge rows rely on zeroed u/d slots (memsets)
                sx = r % SX
                s1 = r % S
                ps = ps1pool.tile([HID_C, 1, W], f32, tag="ps")
                nc.tensor.matmul(
                    ps, w1p[:, :],
                    x30[:, sx : sx + 1, C0 : C0 + W],
                    start=True, stop=True,
                )
                l1_copies(ps, s1, 1)

            def l1_copies(ps, s1, n):
                m = h1[:, 0, s1 : s1 + n, C0 : C0 + W]
                r_ = h1[:, 1, s1 : s1 + n, C0 : C0 + W]
                if not has_bias:
                    nc.scalar.activation(m, ps, Relu, bias=b1_sb)
                    nc.vector.scalar_tensor_tensor(r_, ps, 0.0, m, Max, Sub)
                else:
                    z = zb[0][:, 0:n, :]
                    nc.scalar.activation(z, ps, Relu, bias=b1_sb)
                    nc.vector.tensor_scalar_add(m, z, 0.0)
                    nc.vector.scalar_tensor_tensor(r_, z, 0.0, m, Bypass, Sub)

            def conv_row_mms(r, hsrc, wsb):
                """The 9 DR matmul (lhs, rhs) pairs for one conv row."""
                sg = r % S
                hf = hsrc  # [128, 2, S, WP]
                mms = []
                for t, dr, dc in TAPS:
                    if 0 <= r + dr < H:
                        st = (r + dr) % S
                        mms.append((
                            wsb[:, :, G_MAIN[t], :],
                            hf[:, 0:2, st, C0 + dc : C0 + dc + W],
                        ))
                u_ok = r - 1 >= 0
                d_ok = r + 1 < H
                if u_ok and sg >= 1:
                    mms.append((
                        wsb[:, :, G_WLO_A, :],
                        hf[:, 0, sg - 1 : sg + 1, C0 : C0 + W],
                    ))
                else:
                    if u_ok:
                        mms.append((
                            wsb[:, :, G_WLO_US, :],
                            hf[:, 0:2, (sg - 1) % S, C0 : C0 + W],
                        ))
                    mms.append((
                        wsb[:, :, G_WLO_CS, :],
                        hf[:, 0:2, sg, C0 : C0 + W],
                    ))
                if d_ok:
                    mms.append((
                        wsb[:, :, G_WLO_B, :],
                        hf[:, 0:2, (sg + 1) % S, C0 : C0 + W],
                    ))
                mms.append((
                    wsb[:, :, G_WLO_C, :],
                    hf[:, 0:2, sg, C0 - 1 : C0 - 1 + W],
                ))
                mms.append((
                    wsb[:, :, G_WLO_D, :],
                    hf[:, 0:2, sg, C0 + 1 : C0 + 1 + W],
                ))
                return mms

            def emit_conv_pair(r0, hsrc, wsb, dst_m, dst_r, bias, scale,
                               zbi, pspool):
                """Rows (r0, r0+1) of a 128->128 conv; shared psum tile."""
                if KSTAGES < (2 if dst_r is not None else 3):
                    return
                sg = r0 % S
                ps = pspool.tile([HID_C, 2, W], f32, tag="ps")
                for i in range(2):
                    mms = conv_row_mms(r0 + i, hsrc, wsb)
                    for j, (lhs, rhs) in enumerate(mms):
                        nc.tensor.matmul(
                            ps[:, i, :], lhs, rhs,
                            start=(j == 0), stop=(j == len(mms) - 1),
                            perf_mode=DR,
                        )
                if dst_r is None:
                    # h3: single bf16 plane, free scale
                    nc.scalar.activation(
                        dst_m[:, sg : sg + 2, C0 : C0 + W], ps, Relu,
                        bias=bias, scale=scale)
                    return
                m = dst_m[:, 0, sg : sg + 2, C0 : C0 + W]
                r_ = dst_m[:, 1, sg : sg + 2, C0 : C0 + W]
                if not has_bias:
                    nc.scalar.activation(m, ps, Relu, bias=bias)
                    nc.vector.scalar_tensor_tensor(r_, ps, 0.0, m, Max, Sub)
                else:
                    z = zb[zbi][:, 0:2, :]
                    nc.scalar.activation(z, ps, Relu, bias=bias)
                    nc.vector.tensor_scalar_add(m, z, 0.0)
                    nc.vector.scalar_tensor_tensor(r_, z, 0.0, m, Bypass, Sub)

            l4a_ct = [0]

            def emit_l4a(r0, n, a, tcur):
                if KSTAGES < 4:
                    return
                s3 = r0 % S
                if s3 + n > S:  # slot wrap mid-chunk
                    for r in range(r0, r0 + n):
                        emit_l4a(r, 1, a, tcur)
                    return
                ps = ps4pool.tile([HID_C, n, W], f32, tag="ps")
                nc.tensor.matmul(
                    ps, w4a[:, :], h3[:, s3 : s3 + n, C0 : C0 + W],
                    start=True, stop=True,
                )
                d = r0 - (a - 1)
                l4a_ct[0] += 1
                if l4a_ct[0] % 2 == 0:
                    nc.vector.tensor_scalar_add(
                        tcur[:, d : d + n, C0 : C0 + W], ps, t5b_sb)
                else:
                    nc.scalar.activation(
                        tcur[:, d : d + n, C0 : C0 + W], ps, Ident,
                        bias=t5b_sb)

            def emit_gathers(a, ra, rb, tcur, first, last):
                # y rows [ra, rb) of the strip starting at a.
                # center: plain copy initializing this half's accum bytes;
                # 3-D APs keep descriptor runs ~1KB (swdge accum limit)
                c0r = 0 if first else ra - a + 1
                c1r = T5R if last else rb - a + 1
                nc.gpsimd.dma_start(
                    out=yacc[:, c0r:c1r, :],
                    in_=tcur[SLAB["c"] : SLAB["c"] + 6, c0r:c1r, :],
                )
                # the 4 shifted taps accumulate (cast bf16 -> f32)
                for t, dr, dc in TAPS:
                    if t == "c":
                        continue
                    r0 = max(ra, -dr) if dr < 0 else ra
                    r1 = min(rb, H - dr) if dr > 0 else rb
                    d0 = r0 - a + 1
                    d1 = r1 - a + 1
                    nc.gpsimd.dma_start(
                        out=yacc[:, d0:d1, C0 : C0 + W],
                        in_=tcur[SLAB[t] : SLAB[t] + 6, d0 + dr : d1 + dr,
                                 C0 + dc : C0 + dc + W],
                        accum_op=Add,
                    )
                nc.sync.dma_start(
                    out=y_d[:, ra:rb, :],
                    in_=yacc[:, ra - a + 1 : rb - a + 1, C0 : C0 + W],
                )

            # ---------- schedule ----------
            cur1 = cur2 = cur3 = 0
            x_loads(0)
            for k in range(NS):
                a, b = k * R, k * R + R
                if k == NS - 1:
                    # d-group slot for x[256] (stale from strip 5); row 255
                    nc.sync.dma_start(
                        out=x30[24:30, (H - 1) % SX : (H - 1) % SX + 1, :],
                        in_=zeros_bf[:, 0:WP])
                t1 = min(H, b + 6)
                t2 = min(H, b + 4)
                t3 = min(H, b + 2)
                tcur = t5[k % 2]
                cur4 = max(0, a - 1)
                t4 = min(H, b + 1)
                xl_done = False
                while cur1 < t1 or cur2 < t2 or cur3 < t3 or cur4 < t4:
                    prog = False
                    if not xl_done and cur1 >= t1 and k + 1 < NS:
                        x_loads(k + 1)
                        xl_done = True
                    if cur1 < t1:
                        n = min(2, t1 - cur1)
                        emit_l1(cur1, n)
                        cur1 += n
                        prog = True
                    if (cur2 < t2 and (cur2 + 3 + LAG <= cur1
                                       or cur1 >= t1)):
                        emit_conv_pair(
                            cur2, h1, w2p, h2, h2, b2s_sb, 1.0, 1, ps2pool)
                        cur2 += 2
                        prog = True
                    if (cur3 < t3 and (cur3 + 3 + LAG <= cur2
                                       or cur2 >= t2)):
                        emit_conv_pair(
                            cur3, h2, w3p, h3, None, b3_sb, 2.0 ** -10, 2,
                            ps3pool)
                        cur3 += 2
                        prog = True
                    if cur4 < t4 and (cur4 + min(2, t4 - cur4) + LAG <= cur3
                                      or cur3 >= t3):
                        n = min(2, t4 - cur4)
                        emit_l4a(cur4, n, a, tcur)
                        cur4 += n
                        prog = True
                    assert prog, (k, cur1, cur2, cur3, cur4)
                if not xl_done and k + 1 < NS:
                    x_loads(k + 1)
                emit_gathers(a, a, b, tcur, True, True)

    nc.finalize()
    return nc


_NC_CACHE = {}


def _pack_inputs(x, w1, b1, w2, b2, w3, b3, w4, b4):
    e4 = ml_dtypes.float8_e4m3
    x = np.asarray(x, dtype=np.float32)
    w1 = np.asarray(w1, dtype=np.float32)
    w2 = np.asarray(w2, dtype=np.float32)
    w3 = np.asarray(w3, dtype=np.float32)
    w4 = np.asarray(w4, dtype=np.float32)
    b1 = np.asarray(b1, np.float32)
    b2 = np.asarray(b2, np.float32)
    b3 = np.asarray(b3, np.float32)
    b4 = np.asarray(b4, np.float32)

    # w index order in reference: 0=center,1=up,2=down,3=left,4=right
    TAP_W = {"c": 0, "u": 1, "d": 2, "l": 3, "r": 4}

    # L1: [30, 128] bf16, groups u,c,l,r,d (partition-droppable edges)
    w1p = np.zeros((30, HID_C), np.float32)
    for g, t in enumerate(["u", "c", "l", "r", "d"]):
        w1p[6 * g : 6 * g + 6, :] = w1[:, :, TAP_W[t]].T
    w1p = w1p.astype(ml_dtypes.bfloat16)

    def pack_dr(wl):
        """[128, 2, 11, 128] fp8: main pairs + Wlo groups."""
        ws = wl * SW  # [oc, ic, 5]
        hi = ws.astype(e4).astype(np.float32)
        lo = (ws - hi).astype(e4).astype(np.float32)
        out = np.zeros((HID_C, 2, 11, HID_C), np.float32)
        for t, g in G_MAIN.items():
            whi = hi[:, :, TAP_W[t]].T  # [ic(K), oc(M)]
            out[:, 0, g, :] = whi
            out[:, 1, g, :] = whi
        lo_t = {t: lo[:, :, TAP_W[t]].T for t in TAP_W}
        out[:, 0, G_WLO_A, :] = lo_t["u"]
        out[:, 1, G_WLO_A, :] = lo_t["c"]
        out[:, 0, G_WLO_B, :] = lo_t["d"]
        out[:, 0, G_WLO_C, :] = lo_t["l"]
        out[:, 0, G_WLO_D, :] = lo_t["r"]
        out[:, 0, G_WLO_CS, :] = lo_t["c"]
        out[:, 0, G_WLO_US, :] = lo_t["u"]
        return out.astype(e4)

    # L4 slabs: u@0, c@6, d@32, l@64, r@96
    w4a = np.zeros((HID_C, HID_C), np.float32)
    for t, p in SLAB.items():
        w4a[:, p : p + OUT_C] = w4[:, :, TAP_W[t]].T
    t5b = np.zeros((HID_C,), np.float32)
    t5b[SLAB["c"] : SLAB["c"] + OUT_C] = b4

    common = {
        "w1p": w1p,
        "w2p": pack_dr(w2),
        "w3p": pack_dr(w3),
        "w4a": w4a.astype(ml_dtypes.bfloat16),
        "b1": b1,
        "b2s": b2 * SW,
        "b3": b3,
        "t5b": t5b,
    }
    has_bias = any(
        float(np.abs(v).max()) > 0 for v in (b1, b2, b3))
    return x.astype(ml_dtypes.bfloat16), common, has_bias


def kernel(x, w1, b1, w2, b2, w3, b3, w4, b4):
    x, common, has_bias = _pack_inputs(x, w1, b1, w2, b2, w3, b3, w4, b4)
    key = ("nc", has_bias)
    if key not in _NC_CACHE:
        _NC_CACHE[key] = _build(has_bias)
    _NC_CACHE["nc"] = _NC_CACHE[key]
    nc = _NC_CACHE[key]
    in_maps = [dict(common, x=np.ascontiguousarray(x[i])) for i in range(N_CORES)]
    res = bass_utils.run_bass_kernel_spmd(nc, in_maps, core_ids=list(range(N_CORES)))
    out = np.stack([res.results[i]["y"] for i in range(N_CORES)], axis=0)
    return out
